# revision 9
# baseline (speedup 1.0000x reference)
"""Conv7x7(SAME) + LIF scan kernel for Trainium2, 8 NeuronCores.

Strategy (v2):
- Shard W=512 across cores: core c owns output cols [64c, 64c+64), receives a
  70-col slab (3-col halo each side, zero-padded) over all 512 rows and all
  128 timesteps, H-padded to 518 rows, laid out [518, 128, 70] in DRAM.
- Conv: contract over H on the TensorEngine. Stationary = banded matrix
  B[k, m] = W'[k-m, dx] mapping 128 input rows -> 122 output rows; the 512
  output rows split into 5 blocks at stride 122 (last block 24 valid rows).
  All 5 blocks merge into ONE matmul per tap via a 3D moving AP
  [128, (block, 64)], psum [122, 5*64=320]. 7 dx taps accumulate in PSUM.
- Precision: fp32 matmul costs 4 cycles/row; fp16/bf16 cost 1. The LIF spike
  cascade needs ~fp32 conv precision (bf16/tf32/fp32r all flip too many
  spikes), so the conv runs as a 3-term fp16 hi/lo decomposition
  (wh@xh + wh@xl + wl@xh), measured at ~1.5e-7 max abs error vs fp32.
  21 matmuls/step of ap 320 ~= 2.8us/step on the PE.
- LIF (4 DVE ops/step, tracking j = 0.1*i so the 0.1 folds into W'):
    u = 0.9*v + j ; m = (u <= 1) [bf16, the DMA'd output] ; v = u*m ;
    j = 0.9*j + psum. Host computes z = 1 - m.
- Input preloads in 16-step chunks (ring of 6); output masks batch 8 steps
  per DMA as raw [122, 2560] mega-tiles the host unscrambles.
"""
import numpy as np
import concourse.bacc as bacc
import concourse.mybir as mybir
import concourse.tile as tile
from concourse.bass_utils import run_bass_kernel_spmd

T, H, WD, KK, PAD = 128, 512, 512, 7, 3
NCORES = 8
COLS = WD // NCORES           # 64 output cols per core
KP = COLS + 2 * PAD           # 70 input cols per core
NB = 5                        # row blocks per step
BS = 122                      # output rows per block (contract 128, 7-tap)
HP = 4 * BS + 128             # 616 slab rows: rows beyond 518 are host zeros
                              # so every block DMAs a full 128 rows, no memsets
CK = 16                       # max timesteps per input chunk (tile capacity)
CHUNKS = [4, 4, 8] + [16] * 7             # sizes; small head to start PE early
CSTART = [sum(CHUNKS[:i]) for i in range(len(CHUNKS))]
RING = 6                      # resident chunk ring
GB = 4                        # timesteps per output mega-DMA
NG = T // GB

_cached = None


def _build():
    global _cached
    if _cached is not None:
        return _cached

    f32 = mybir.dt.float32
    f16 = mybir.dt.float16
    bf16 = mybir.dt.bfloat16
    Alu = mybir.AluOpType

    nc = bacc.Bacc("TRN2", debug=False, num_devices=NCORES)
    xh_d = nc.dram_tensor("xh", (HP, T, KP), f16, kind="ExternalInput")
    xl_d = nc.dram_tensor("xl", (HP, T, KP), f16, kind="ExternalInput")
    bmh_d = nc.dram_tensor("bmh", (128, KK * BS), f16, kind="ExternalInput")
    bml_d = nc.dram_tensor("bml", (128, KK * BS), f16, kind="ExternalInput")
    ms_d = nc.dram_tensor("ms", (NG, BS, GB * NB * 64), bf16,
                          kind="ExternalOutput")

    CW = CK * KP              # 1120 cols per block in a chunk tile

    with tile.TileContext(nc) as tc:
        with (
            tc.tile_pool(name="pool", bufs=1) as pool,
            tc.tile_pool(name="psum", bufs=1, space="PSUM") as psum,
        ):
            bmh_t = pool.tile([128, KK * BS], f16, name="bmh")
            bml_t = pool.tile([128, KK * BS], f16, name="bml")
            nc.sync.dma_start(bmh_t[:], bmh_d.ap())
            nc.sync.dma_start(bml_t[:], bml_d.ap())

            xhc = [pool.tile([128, NB * CW], f16, name=f"xh{r}")
                   for r in range(RING)]
            xlc = [pool.tile([128, NB * CW], f16, name=f"xl{r}")
                   for r in range(RING)]
            u_t = pool.tile([128, NB * 64], f32, name="u")
            v_t = pool.tile([128, NB * 64], f32, name="v")
            j_t = pool.tile([128, NB * 64], f32, name="j")
            nc.vector.memset(v_t[:], 0.0)
            nc.vector.memset(j_t[:], 0.0)

            mg = [pool.tile([128, GB * NB * 64], bf16, name=f"mg{i}")
                  for i in range(2)]
            pss = [psum.tile([128, NB * 64], f32, name=f"ps{i}")
                   for i in range(8)]

            in_eng = [nc.sync, nc.scalar]
            n_in = [0]

            def load_chunk(ci):
                r = ci % RING
                sz = CHUNKS[ci]
                t0 = CSTART[ci]
                for dst, src in ((xhc[r], xh_d), (xlc[r], xl_d)):
                    for b in range(NB):
                        eng = in_eng[n_in[0] % len(in_eng)]
                        n_in[0] += 1
                        eng.dma_start(
                            dst[:, b * CW:b * CW + sz * KP]
                            .rearrange("p (t q) -> p t q", q=KP),
                            src.ap()[BS * b:BS * b + 128, t0:t0 + sz, :])

            for ci in range(3):
                load_chunk(ci)

            # PE pstate warmup: harmless self-matmuls while preload streams in
            for i in range(6):
                nc.tensor.matmul(pss[7][0:BS, :], bmh_t[:, 0:BS],
                                 bmh_t[:, 0:320], start=True, stop=True)

            step2chunk = []
            for ci, sz in enumerate(CHUNKS):
                step2chunk += [(ci, tl) for tl in range(sz)]

            for t in range(T):
                ck, tl = step2chunk[t]
                if tl == 0 and ck + 3 < len(CHUNKS):
                    load_chunk(ck + 3)
                r = ck % RING
                mvh = xhc[r][:, :].rearrange(
                    "p (b t q) -> p b t q", b=NB, t=CK)
                mvl = xlc[r][:, :].rearrange(
                    "p (b t q) -> p b t q", b=NB, t=CK)
                ps = pss[t % 8]
                n = 0
                for dx in range(KK):
                    for bm_t, mv in ((bmh_t, mvh), (bmh_t, mvl),
                                     (bml_t, mvh)):
                        nc.tensor.matmul(
                            ps[0:BS, :],
                            bm_t[:, dx * BS:(dx + 1) * BS],
                            mv[:, :, tl:tl + 1, dx:dx + 64],
                            start=(n == 0), stop=(n == 3 * KK - 1),
                        )
                        n += 1

                msl = mg[(t // GB) % 2][0:BS, (t % GB) * 320:(t % GB + 1) * 320]
                nc.vector.scalar_tensor_tensor(
                    u_t[0:BS, :], v_t[0:BS, :], 0.9, j_t[0:BS, :],
                    Alu.mult, Alu.add)
                nc.vector.tensor_scalar(
                    msl, u_t[0:BS, :], 1.0, None, Alu.is_le)
                nc.vector.tensor_tensor(
                    v_t[0:BS, :], u_t[0:BS, :], msl, Alu.mult)
                nc.vector.scalar_tensor_tensor(
                    j_t[0:BS, :], j_t[0:BS, :], 0.9, ps[0:BS, :],
                    Alu.mult, Alu.add)

                if t % GB == GB - 1:
                    g = t // GB
                    nc.gpsimd.dma_start(ms_d.ap()[g], mg[g % 2][0:BS, :])

    nc.compile()
    _cached = nc
    return nc


def _bands(Wq):
    bm = np.zeros((128, KK * BS), np.float32)
    m = np.arange(BS)
    for dx in range(KK):
        for dy in range(KK):
            bm[m + dy, dx * BS + m] = Wq[dy, dx]
    return bm.astype(np.float16)


def kernel(x, W):
    x = np.asarray(x, np.float32)
    nc = _build()

    W01 = (np.float32(0.1) * np.asarray(W, np.float32).reshape(KK, KK))
    wh = W01.astype(np.float16)
    wl = (W01 - wh.astype(np.float32)).astype(np.float16)
    bmh = _bands(wh.astype(np.float32))
    bml = _bands(wl.astype(np.float32))

    in_maps = []
    for c in range(NCORES):
        slab = np.zeros((T, HP, KP), np.float32)
        lo = COLS * c - PAD
        s0, s1 = max(0, lo), min(WD, lo + KP)
        slab[:, PAD:PAD + H, s0 - lo:s0 - lo + s1 - s0] = x[:, 0, :, s0:s1]
        xs = np.ascontiguousarray(slab.transpose(1, 0, 2))   # [616, 128, 70]
        xh = xs.astype(np.float16)
        xl = (xs - xh.astype(np.float32)).astype(np.float16)
        in_maps.append({"xh": xh, "xl": xl, "bmh": bmh, "bml": bml})

    res = run_bass_kernel_spmd(nc, in_maps, core_ids=list(range(NCORES)))

    z = np.empty((T, H, WD), np.float32)
    for c in range(NCORES):
        ms = np.asarray(res.results[c]["ms"]).astype(np.float32)
        # [NG, BS, GB*NB*64] -> [t, block, row, w]
        m4 = ms.reshape(NG, BS, GB, NB, 64).transpose(0, 2, 3, 1, 4)
        m4 = m4.reshape(T, NB, BS, 64)
        zc = z[:, :, COLS * c:COLS * (c + 1)]
        zc[:, 0:4 * BS, :] = m4[:, 0:4].reshape(T, 4 * BS, 64)
        zc[:, 4 * BS:H, :] = m4[:, 4, 0:H - 4 * BS, :]
    return (np.float32(1.0) - z).reshape(T, 1, H, WD)


# revision 10
# speedup vs baseline: 1.0137x; 1.0137x over previous
"""Conv7x7(SAME) + LIF scan kernel for Trainium2, 8 NeuronCores.

Strategy (v2):
- Shard W=512 across cores: core c owns output cols [64c, 64c+64), receives a
  70-col slab (3-col halo each side, zero-padded) over all 512 rows and all
  128 timesteps, H-padded to 518 rows, laid out [518, 128, 70] in DRAM.
- Conv: contract over H on the TensorEngine. Stationary = banded matrix
  B[k, m] = W'[k-m, dx] mapping 128 input rows -> 122 output rows; the 512
  output rows split into 5 blocks at stride 122 (last block 24 valid rows).
  All 5 blocks merge into ONE matmul per tap via a 3D moving AP
  [128, (block, 64)], psum [122, 5*64=320]. 7 dx taps accumulate in PSUM.
- Precision: fp32 matmul costs 4 cycles/row; fp16/bf16 cost 1. The LIF spike
  cascade needs ~fp32 conv precision (bf16/tf32/fp32r all flip too many
  spikes), so the conv runs as a 3-term fp16 hi/lo decomposition
  (wh@xh + wh@xl + wl@xh), measured at ~1.5e-7 max abs error vs fp32.
  21 matmuls/step of ap 320 ~= 2.8us/step on the PE.
- LIF (4 DVE ops/step, tracking j = 0.1*i so the 0.1 folds into W'):
    u = 0.9*v + j ; m = (u <= 1) [bf16, the DMA'd output] ; v = u*m ;
    j = 0.9*j + psum. Host computes z = 1 - m.
- Input preloads in 16-step chunks (ring of 6); output masks batch 8 steps
  per DMA as raw [122, 2560] mega-tiles the host unscrambles.
"""
import numpy as np
import concourse.bacc as bacc
import concourse.mybir as mybir
import concourse.tile as tile
from concourse.bass_utils import run_bass_kernel_spmd

T, H, WD, KK, PAD = 128, 512, 512, 7, 3
NCORES = 8
COLS = WD // NCORES           # 64 output cols per core
KP = COLS + 2 * PAD           # 70 input cols per core
NB = 5                        # row blocks per step
BS = 122                      # output rows per block (contract 128, 7-tap)
HP = 4 * BS + 128             # 616 slab rows: rows beyond 518 are host zeros
                              # so every block DMAs a full 128 rows, no memsets
CK = 16                       # max timesteps per input chunk (tile capacity)
CHUNKS = [4, 4, 8] + [16] * 7             # sizes; small head to start PE early
CSTART = [sum(CHUNKS[:i]) for i in range(len(CHUNKS))]
RING = 6                      # resident chunk ring
GB = 4                        # timesteps per output mega-DMA
NG = T // GB

_cached = None


def _build():
    global _cached
    if _cached is not None:
        return _cached

    f32 = mybir.dt.float32
    f16 = mybir.dt.float16
    bf16 = mybir.dt.bfloat16
    Alu = mybir.AluOpType

    nc = bacc.Bacc("TRN2", debug=False, num_devices=NCORES)
    xh_d = nc.dram_tensor("xh", (HP, T, KP), f16, kind="ExternalInput")
    xl_d = nc.dram_tensor("xl", (HP, T, KP), f16, kind="ExternalInput")
    bmh_d = nc.dram_tensor("bmh", (128, KK * BS), f16, kind="ExternalInput")
    bml_d = nc.dram_tensor("bml", (128, KK * BS), f16, kind="ExternalInput")
    ms_d = nc.dram_tensor("ms", (NG, BS, GB * NB * 64), bf16,
                          kind="ExternalOutput")

    CW = CK * KP              # 1120 cols per block in a chunk tile

    with tile.TileContext(nc) as tc:
        with (
            tc.tile_pool(name="pool", bufs=1) as pool,
            tc.tile_pool(name="psum", bufs=1, space="PSUM") as psum,
        ):
            wu_t = pool.tile([128, NB * 64], f16, name="wu")
            nc.vector.memset(wu_t[:], 0.0)
            bmh_t = pool.tile([128, KK * BS], f16, name="bmh")
            bml_t = pool.tile([128, KK * BS], f16, name="bml")
            nc.sync.dma_start(bmh_t[:], bmh_d.ap())
            nc.sync.dma_start(bml_t[:], bml_d.ap())

            xhc = [pool.tile([128, NB * CW], f16, name=f"xh{r}")
                   for r in range(RING)]
            xlc = [pool.tile([128, NB * CW], f16, name=f"xl{r}")
                   for r in range(RING)]
            u_t = pool.tile([128, NB * 64], f32, name="u")
            v_t = pool.tile([128, NB * 64], f32, name="v")
            j_t = pool.tile([128, NB * 64], f32, name="j")
            nc.vector.memset(v_t[:], 0.0)
            nc.vector.memset(j_t[:], 0.0)

            mg = [pool.tile([128, GB * NB * 64], bf16, name=f"mg{i}")
                  for i in range(2)]
            pss = [psum.tile([128, NB * 64], f32, name=f"ps{i}")
                   for i in range(8)]

            in_eng = [nc.sync, nc.scalar]
            n_in = [0]

            def load_chunk(ci):
                r = ci % RING
                sz = CHUNKS[ci]
                t0 = CSTART[ci]
                for dst, src in ((xhc[r], xh_d), (xlc[r], xl_d)):
                    for b in range(NB):
                        eng = in_eng[n_in[0] % len(in_eng)]
                        n_in[0] += 1
                        eng.dma_start(
                            dst[:, b * CW:b * CW + sz * KP]
                            .rearrange("p (t q) -> p t q", q=KP),
                            src.ap()[BS * b:BS * b + 128, t0:t0 + sz, :])

            for ci in range(3):
                load_chunk(ci)

            # PE pstate warmup: harmless self-matmuls while preload streams in
            # (zero wu tile, no DMA dependency, so the PE starts ASAP)
            for i in range(6):
                nc.tensor.matmul(pss[7][0:BS, :], wu_t[:, 0:BS],
                                 wu_t[:, :], start=True, stop=True)

            step2chunk = []
            for ci, sz in enumerate(CHUNKS):
                step2chunk += [(ci, tl) for tl in range(sz)]

            for t in range(T):
                ck, tl = step2chunk[t]
                if tl == 0 and ck + 3 < len(CHUNKS):
                    load_chunk(ck + 3)
                r = ck % RING
                mvh = xhc[r][:, :].rearrange(
                    "p (b t q) -> p b t q", b=NB, t=CK)
                mvl = xlc[r][:, :].rearrange(
                    "p (b t q) -> p b t q", b=NB, t=CK)
                ps = pss[t % 8]
                n = 0
                for dx in range(KK):
                    for bm_t, mv in ((bmh_t, mvh), (bmh_t, mvl),
                                     (bml_t, mvh)):
                        nc.tensor.matmul(
                            ps[0:BS, :],
                            bm_t[:, dx * BS:(dx + 1) * BS],
                            mv[:, :, tl:tl + 1, dx:dx + 64],
                            start=(n == 0), stop=(n == 3 * KK - 1),
                        )
                        n += 1

                msl = mg[(t // GB) % 2][0:BS, (t % GB) * 320:(t % GB + 1) * 320]
                nc.vector.scalar_tensor_tensor(
                    u_t[0:BS, :], v_t[0:BS, :], 0.9, j_t[0:BS, :],
                    Alu.mult, Alu.add)
                nc.vector.tensor_scalar(
                    msl, u_t[0:BS, :], 1.0, None, Alu.is_le)
                nc.vector.tensor_tensor(
                    v_t[0:BS, :], u_t[0:BS, :], msl, Alu.mult)
                nc.vector.scalar_tensor_tensor(
                    j_t[0:BS, :], j_t[0:BS, :], 0.9, ps[0:BS, :],
                    Alu.mult, Alu.add)

                if t == T - 3:
                    # last group: flush its first half early to shorten the
                    # post-compute DMA drain tail
                    nc.gpsimd.dma_start(ms_d.ap()[NG - 1][:, 0:640],
                                        mg[(NG - 1) % 2][0:BS, 0:640])
                if t % GB == GB - 1:
                    g = t // GB
                    if g == NG - 1:
                        nc.gpsimd.dma_start(ms_d.ap()[g][:, 640:],
                                            mg[g % 2][0:BS, 640:])
                    else:
                        nc.gpsimd.dma_start(ms_d.ap()[g], mg[g % 2][0:BS, :])

    nc.compile()
    _cached = nc
    return nc


def _bands(Wq):
    bm = np.zeros((128, KK * BS), np.float32)
    m = np.arange(BS)
    for dx in range(KK):
        for dy in range(KK):
            bm[m + dy, dx * BS + m] = Wq[dy, dx]
    return bm.astype(np.float16)


def kernel(x, W):
    x = np.asarray(x, np.float32)
    nc = _build()

    W01 = (np.float32(0.1) * np.asarray(W, np.float32).reshape(KK, KK))
    wh = W01.astype(np.float16)
    wl = (W01 - wh.astype(np.float32)).astype(np.float16)
    bmh = _bands(wh.astype(np.float32))
    bml = _bands(wl.astype(np.float32))

    in_maps = []
    for c in range(NCORES):
        slab = np.zeros((T, HP, KP), np.float32)
        lo = COLS * c - PAD
        s0, s1 = max(0, lo), min(WD, lo + KP)
        slab[:, PAD:PAD + H, s0 - lo:s0 - lo + s1 - s0] = x[:, 0, :, s0:s1]
        xs = np.ascontiguousarray(slab.transpose(1, 0, 2))   # [616, 128, 70]
        xh = xs.astype(np.float16)
        xl = (xs - xh.astype(np.float32)).astype(np.float16)
        in_maps.append({"xh": xh, "xl": xl, "bmh": bmh, "bml": bml})

    res = run_bass_kernel_spmd(nc, in_maps, core_ids=list(range(NCORES)))

    z = np.empty((T, H, WD), np.float32)
    for c in range(NCORES):
        ms = np.asarray(res.results[c]["ms"]).astype(np.float32)
        # [NG, BS, GB*NB*64] -> [t, block, row, w]
        m4 = ms.reshape(NG, BS, GB, NB, 64).transpose(0, 2, 3, 1, 4)
        m4 = m4.reshape(T, NB, BS, 64)
        zc = z[:, :, COLS * c:COLS * (c + 1)]
        zc[:, 0:4 * BS, :] = m4[:, 0:4].reshape(T, 4 * BS, 64)
        zc[:, 4 * BS:H, :] = m4[:, 4, 0:H - 4 * BS, :]
    return (np.float32(1.0) - z).reshape(T, 1, H, WD)
